# revision 27
# baseline (speedup 1.0000x reference)
"""BlockAttention TRN2 Bass kernel (algebraically fused, all-fp16 matmuls).

Problem (hardcoded): x [4, 4096, 1024] fp32; wq/wk/wv/wo [1024, 1024];
bq/bk/bv/bo [1024]; block_size 256. Output [4, 8192, 1024]:
per 256-token block g: rows [512g, 512g+256) = softmax(Q_g K_g^T / 32) V_g @ wo,
rows [512g+256, 512g+512) = softmax(Q_g K_{g-1}^T / 32) V_{g-1} @ wo (block 0
attends to itself), all + bo.

Sharding: 8 cores = 4 batches x 2 sequence halves (8 q-blocks each). Each core
gets x^T for its 9 kv blocks (prev + 8 own; block 0's "prev" is itself), the
fused weights, and writes out^T [1024, 4096] bf16 for its 4096 output rows.

Algebraic fusion (host precomputes, exact):
  - Aqk = wq @ wk^T: scores S[q,k] = x_q^T Aqk x_k (+ per-key bias term
    kb[k] = x_k.(wk bq) + bq.bk, softmax-invariant per-query terms dropped).
    One projection QA = (x @ Aqk)^T replaces the Q and K projections, and the
    score matmuls contract QA against the raw x^T tiles already in SBUF.
  - Awv = wv @ wo: VW = x @ Awv replaces V-proj + V@wo. The bv part:
    P_norm @ (1 bv^T wo) = bv@wo (softmax rows sum to 1), added on host
    with bo.

Why fp16 everywhere (measured on this data): fp8e4m3 DoubleRow is 2x the
matmul rate, but at concentrated-attention queries (max softmax weight ~0.4)
a 4% fp8 quantization of VW or P lands directly on the output: ~4e-2 rel
error vs the 2e-2 budget. fp16 matmuls (full PE rate, 10-bit mantissa) give
2.6e-3. PSUM accumulation is fp32 throughout. Accumulation groups sharing a
PSUM bank always run back-to-back; interleaving two open groups in one bank
corrupts the first group's start on hardware.

Per-core algorithm:
  - QA^T = Aqk^T x^T per q-block PAIR (512-wide moving operand amortizes
    the per-matmul weight-load bubble).
  - VW = x @ Awv per kv block ([token-partitions, d_out] layout), reused by
    the local attention of block g and the cross attention of block g+1.
  - S^T [keys, queries] = X_kv QA^T (no transposes anywhere); softmax over
    the partition (key) dim: exp on ScalarE (per-key bias, fp16 out),
    key-sums via ones-vector matmul, reciprocal on VectorE, normalization
    deferred to the PSUM->SBUF output copy (diag scaling commutes with V@wo).
  - out^T = VW^T P^T in PSUM, staged [128, 512] bf16 per (m, q-block) so
    local|cross form one contiguous DMA row pair, then straight to DRAM.
"""

import numpy as np
import ml_dtypes
from contextlib import ExitStack

import concourse.bass as bass
import concourse.mybir as mybir
import concourse.tile as tile
from concourse import bacc, bass_utils

D = 1024
BS = 256
NBQ = 8  # q-blocks per core
NKV = NBQ + 1  # kv blocks (prev + own 8)
DS = D // 128  # 8 subtiles of the feature dim
F32 = mybir.dt.float32
F32R = mybir.dt.float32r
BF16 = mybir.dt.bfloat16
FP16 = mybir.dt.float16
SCALE = 1.0 / 32.0  # 1/sqrt(D)

_CACHED_NC = None


def _build():
    nc = bacc.Bacc("TRN2", target_bir_lowering=False, debug=False, num_devices=8)
    # block-tiled fp16 x^T: row 128*b+p, col 256*s+c = x_kv[256b+c, 128s+p]
    x16 = nc.dram_tensor("x16", [NKV * 128, DS * BS], FP16, kind="ExternalInput").ap()
    aqk = nc.dram_tensor("aqk", [128, DS * D], FP16, kind="ExternalInput").ap()
    awv = nc.dram_tensor("awv", [128, DS * D], FP16, kind="ExternalInput").ap()
    kb = nc.dram_tensor("kb", [128, NKV * 2], F32, kind="ExternalInput").ap()
    ones16 = nc.dram_tensor("ones16", [128, 128], FP16, kind="ExternalInput").ap()
    outt = nc.dram_tensor("outt", [D, NBQ * 2 * BS], BF16, kind="ExternalOutput").ap()

    with (
        tile.TileContext(nc) as tc,
        ExitStack() as ctx,
        nc.allow_low_precision(reason="fp16 matmul inputs by design"),
    ):
        wp = ctx.enter_context(tc.tile_pool(name="wp", bufs=1))
        cp = ctx.enter_context(tc.tile_pool(name="cp", bufs=1))
        xp16 = ctx.enter_context(tc.tile_pool(name="xp16", bufs=3))
        qp = ctx.enter_context(tc.tile_pool(name="qp", bufs=2))
        wvp = ctx.enter_context(tc.tile_pool(name="wvp", bufs=3))
        pp = ctx.enter_context(tc.tile_pool(name="pp", bufs=4))
        rp = ctx.enter_context(tc.tile_pool(name="rp", bufs=4))
        sp_ = ctx.enter_context(tc.tile_pool(name="sp", bufs=16))
        PSUM = bass.MemorySpace.PSUM
        ps = ctx.enter_context(tc.tile_pool(name="ps", bufs=8, space=PSUM))

        # fused fp16 weights, [128, DS, D]: [p, s, d] = W[128s+p, d].
        # Split into two DMAs across two rings to halve arrival latency.
        w_sb = {}

        def load_w(name, ap, npkt=2):
            # npkt packets alternating across the two hardware rings; finer
            # packets shrink the region a consumer matmul must wait for.
            t = wp.tile([128, DS, D], FP16, tag=name)
            step = DS // npkt
            for i in range(npkt):
                eng = nc.sync if i % 2 == 0 else nc.scalar
                eng.dma_start(
                    t[:, step * i : step * (i + 1), :],
                    ap[:, step * i * D : step * (i + 1) * D],
                )
            w_sb[name] = t

        def load_x16(b0, nb, engines=None):
            # fp16 x^T tile [128, DS, 2*BS] holding blocks b0, b0+1 (nb=2)
            # or just b0 (nb=1); one 512KB DMA per block, 2KB rows. Steady
            # loads ride the gpsimd ring; startup-critical ones are placed
            # on the fast hardware rings via `engines`.
            t = xp16.tile([128, DS, 2 * BS], FP16, tag="x16")
            for i in range(nb):
                eng = engines[i] if engines else nc.gpsimd
                eng.dma_start(
                    t[:, :, BS * i : BS * (i + 1)],
                    x16[128 * (b0 + i) : 128 * (b0 + i + 1), :],
                )
            return t

        def qa_proj(xpair):
            # (Aqk^T x^T)[d_out, tok] fp16 for a q-block pair: [128, DS, 2*BS],
            # d_out-subtile m at [:, m, :]; 512-wide moving operand.
            dst = qp.tile([128, DS, 2 * BS], FP16, tag="qa")
            for m in range(DS):
                pst = ps.tile([128, 512], F32, tag="ps")
                for s in range(DS):
                    nc.tensor.matmul(
                        pst[:],
                        w_sb["aqk"][:, s, 128 * m : 128 * (m + 1)],
                        xpair[:, s, :],
                        start=(s == 0),
                        stop=(s == DS - 1),
                    )
                nc.scalar.activation(
                    dst[:, m, :],
                    pst[:],
                    mybir.ActivationFunctionType.Identity,
                )
            return dst

        def vw_direct(xtile, kc):
            # (x @ Awv)[tok, d_out] fp16 for block half kc of the pair tile:
            # [128, 2, D]: [p, ts, d] = VW[128*ts+p, d]; 512-wide moving.
            dst = wvp.tile([128, 2, D], FP16, tag="vw")
            for ts in range(2):
                pst = [
                    ps.tile([128, 512], F32, tag="ps", name=f"pst{i}")
                    for i in range(2)
                ]
                for half in range(2):
                    for s in range(DS):
                        nc.tensor.matmul(
                            pst[half][:],
                            xtile[:, s, BS * kc + 128 * ts : BS * kc + 128 * (ts + 1)],
                            w_sb["awv"][:, s, 512 * half : 512 * (half + 1)],
                            start=(s == 0),
                            stop=(s == DS - 1),
                        )
                for half in range(2):
                    # on ScalarE: keeps VectorE free for the reciprocals and
                    # output muls, so attend_out never waits on these casts
                    nc.scalar.activation(
                        dst[:, ts, 512 * half : 512 * (half + 1)],
                        pst[half][:],
                        mybir.ActivationFunctionType.Identity,
                    )
            return dst

        def attend_scores(qa, qc, xkv, kc, kvblk):
            # expS^T = exp(K Q^T / 32 + kbias) fp16, unnormalized.
            # [128, 2, BS]: key-half ks at [:, ks, :]. lhsT = raw x^T key
            # slices from half kc of the x16 pair tile, rhs = qa half qc.
            pst = ps.tile([128, 512], F32, tag="ps")
            ptile = pp.tile([128, 2, BS], FP16, tag="pt")
            for ks in range(2):
                for s in range(DS):
                    nc.tensor.matmul(
                        pst[:, BS * ks : BS * (ks + 1)],
                        xkv[:, s, BS * kc + 128 * ks : BS * kc + 128 * (ks + 1)],
                        qa[:, s, BS * qc : BS * (qc + 1)],
                        start=(s == 0),
                        stop=(s == DS - 1),
                    )
                c = 2 * kvblk + ks
                nc.scalar.activation(
                    ptile[:, ks, :],
                    pst[:, BS * ks : BS * (ks + 1)],
                    mybir.ActivationFunctionType.Exp,
                    scale=SCALE,
                    bias=kb_sb[:, c : c + 1],
                )
            return ptile

        def attend_norm(pt, bc, i):
            # Broadcasted key-sum of one attend into half a shared PSUM bank
            # (sequential groups); 128-lane reciprocal on VectorE. Emitted
            # right after the attend's scores so the slow reciprocal drains
            # under the following PE phases — by the time the output muls
            # need rc, it's long ready and PSUM slots recycle instantly.
            for ks in range(2):
                nc.tensor.matmul(
                    bc[:, BS * i : BS * (i + 1)],
                    ones_sb[:],
                    pt[:, ks, :],
                    start=(ks == 0),
                    stop=(ks == 1),
                )
            rc = rp.tile([128, BS], F32R, tag="rc")
            nc.vector.reciprocal(rc[:], bc[:, BS * i : BS * (i + 1)])
            return rc

        def attend_out(ptile, rc, vw, t, h, stages, drain_half=False):
            # out^T[m-subtile, q] = VW^T P^T (256-deep key contraction in two
            # ks steps). Normalize on the PSUM->SBUF copy into the [128, 512]
            # bf16 stage (local h=0 left, cross h=1 right); the stage row
            # pair is one contiguous DMA per m.
            for mp in range(DS // 2):
                pso = ps.tile([128, 512], F32, tag="ps")
                for sub in range(2):
                    m = 2 * mp + sub
                    for ks in range(2):
                        nc.tensor.matmul(
                            pso[:, BS * sub : BS * (sub + 1)],
                            vw[:, ks, 128 * m : 128 * (m + 1)],
                            ptile[:, ks, :],
                            start=(ks == 0),
                            stop=(ks == 1),
                        )
                for sub in range(2):
                    m = 2 * mp + sub
                    nc.vector.tensor_mul(
                        stages[m][:, BS * h : BS * (h + 1)],
                        pso[:, BS * sub : BS * (sub + 1)],
                        rc[:],
                    )
                    eng = nc.sync if m % 2 else nc.scalar
                    if drain_half:  # final iteration: don't hold the tail
                        eng.dma_start(
                            outt[
                                128 * m : 128 * (m + 1),
                                512 * t + BS * h : 512 * t + BS * (h + 1),
                            ],
                            stages[m][:, BS * h : BS * (h + 1)],
                        )
                    elif h == 0:  # local runs second; both halves written
                        eng.dma_start(
                            outt[128 * m : 128 * (m + 1), 512 * t : 512 * (t + 1)],
                            stages[m][:],
                        )

        # Prologue DMAs on the two fast hardware rings (~110 GB/s each),
        # interleaved per d-subtile in consumption order: the first qa
        # matmul needs only subtile 0 of aqk and of x16 blocks 1+2, so the
        # PE starts as soon as the first three small packets land and rides
        # just behind the rings through the rest. Block 0 (cross keys +
        # vw_prev) and awv are only touched ~20us in. The gpsimd software
        # ring (slow, ~90 GB/s) carries constants + steady x16 prefetch.
        aqk_t = wp.tile([128, DS, D], FP16, tag="aqk")
        w_sb["aqk"] = aqk_t
        x16_cur = xp16.tile([128, DS, 2 * BS], FP16, tag="x16")  # q-blocks 1,2
        for s in range(DS):
            e0, e1 = (nc.sync, nc.scalar) if s % 2 == 0 else (nc.scalar, nc.sync)
            e0.dma_start(aqk_t[:, s, :], aqk[:, s * D : (s + 1) * D])
            e1.dma_start(x16_cur[:, s, 0:BS], x16[128 * 1 : 128 * 2, 256 * s : 256 * (s + 1)])
            # block 2 arrives slightly later than block 1 is consumed; the
            # otherwise-idle gpsimd ring keeps it off the two fast rings
            nc.gpsimd.dma_start(
                x16_cur[:, s, BS : 2 * BS], x16[128 * 2 : 128 * 3, 256 * s : 256 * (s + 1)]
            )
        kb_sb = cp.tile([128, NKV * 2], F32, tag="kb")
        nc.gpsimd.dma_start(kb_sb[:], kb)
        ones_sb = cp.tile([128, 128], FP16, tag="ones")
        nc.gpsimd.dma_start(ones_sb[:], ones16)
        x16_prev = load_x16(0, 1, engines=(nc.scalar,))  # kv block 0
        load_w("awv", awv, npkt=4)

        vw_prev = None
        qa = None
        for t in range(NBQ):
            b = t + 1  # kv block holding this q-block's tokens
            qc = t % 2  # column half within the qa / x16 pair
            if qc == 0:
                qa = qa_proj(x16_cur)
            x16_next = (
                load_x16(b + 1, 2 if b + 2 <= NBQ else 1)
                if qc == 1 and b + 1 <= NBQ
                else None
            )
            bc = ps.tile([128, 512], F32, tag="ps")
            p_loc = attend_scores(qa, qc, x16_cur, qc, b)
            # cross keys: block b-1 = left half of this pair tile (odd t),
            # right half of the previous pair tile (even t>0), or the lone
            # block-0 tile (t=0, which arrives last — all earlier PE work
            # stays off it and off awv).
            if qc == 1:
                p_cross = attend_scores(qa, qc, x16_cur, 0, b - 1)
            else:
                p_cross = attend_scores(qa, qc, x16_prev, 1 if t else 0, b - 1)
            rc_loc = attend_norm(p_loc, bc, 0)
            rc_cross = attend_norm(p_cross, bc, 1)
            vw_cur = vw_direct(x16_cur, qc)
            if t == 0:
                vw_prev = vw_direct(x16_prev, 0)
            stages = [
                sp_.tile([128, 512], BF16, tag="st", name=f"stage{m}")
                for m in range(DS)
            ]
            # cross first: its vw_prev is ready from last iteration, so the
            # PE stream never waits on vw_cur's PSUM->SBUF copies.
            last = t == NBQ - 1
            attend_out(p_cross, rc_cross, vw_prev, t, 1, stages, drain_half=last)
            attend_out(p_loc, rc_loc, vw_cur, t, 0, stages, drain_half=last)
            vw_prev = vw_cur
            if qc == 1:
                x16_prev, x16_cur = x16_cur, x16_next

    nc.compile()
    return nc


def _get_nc():
    global _CACHED_NC
    if _CACHED_NC is None:
        _CACHED_NC = _build()
    return _CACHED_NC


def _make_in_maps(x, wq, bq, wk, bk, wv, bv, wo):
    aqk = (wq @ wk.T).astype(np.float32)
    awv = (wv @ wo).astype(np.float32)
    # weight tiles [128, DS*D]: [p, s*D+d] = W[128s+p, d]
    wtile = lambda w: np.ascontiguousarray(
        w.reshape(DS, 128, D).transpose(1, 0, 2).reshape(128, DS * D),
        np.float16,
    )
    # per-key score bias (exact; zero when bq == 0): kb[tok] = x.(wk bq)+bq.bk
    kbv = (wk @ bq).astype(np.float32)
    kb_full = (x.reshape(-1, D) @ kbv + float(bq @ bk)).reshape(4, -1) * SCALE
    base = {
        "aqk": wtile(aqk),
        "awv": wtile(awv),
        "ones16": np.ones((128, 128), np.float16),
    }
    in_maps = []
    for c in range(8):
        b, t = c // 2, c % 2
        if t == 0:
            xkv = np.concatenate([x[b, 0:BS], x[b, 0 : NBQ * BS]], axis=0)
            kbc = np.concatenate([kb_full[b, 0:BS], kb_full[b, 0 : NBQ * BS]])
        else:
            xkv = x[b, NBQ * BS - BS : 2 * NBQ * BS]
            kbc = kb_full[b, NBQ * BS - BS : 2 * NBQ * BS]
        # block-tiled: row 128*b+p, col 256*s+c = xkv[256*blk+c, 128*s+p]
        in_maps.append(
            {
                **base,
                "x16": np.ascontiguousarray(
                    xkv.reshape(NKV, BS, DS, 128)
                    .transpose(0, 3, 2, 1)
                    .reshape(NKV * 128, DS * BS),
                    np.float16,
                ),
                "kb": np.ascontiguousarray(
                    kbc.reshape(NKV * 2, 128).T, np.float32
                ),
            }
        )
    return in_maps


def _assemble(results, bv, wo, bo):
    out = np.empty((4, 2 * NBQ * 2 * BS, D), np.float32)
    for c in range(8):
        b, t = c // 2, c % 2
        seg = NBQ * 2 * BS  # 4096 output rows per core
        out[b, seg * t : seg * (t + 1), :] = results[c]["outt"].T.astype(np.float32)
    out += (np.asarray(bo, np.float32) + bv @ wo).reshape(1, 1, D)
    return out


def run(x, wq, bq, wk, bk, wv, bv, wo, bo, trace=False):
    nc = _get_nc()
    in_maps = _make_in_maps(x, wq, bq, wk, bk, wv, bv, wo)
    res = bass_utils.run_bass_kernel_spmd(
        nc, in_maps, core_ids=list(range(8)), trace=trace
    )
    return _assemble(res.results, bv, wo, bo), res


def kernel(x, wq, bq, wk, bk, wv, bv, wo, bo, block_size):
    assert int(block_size) == BS
    x = np.asarray(x, np.float32)
    assert x.shape == (4, 2 * NBQ * BS, D), x.shape
    args = [np.asarray(a, np.float32) for a in (wq, bq, wk, bk, wv, bv, wo, bo)]
    wq, bq, wk, bk, wv, bv, wo, bo = args
    out, _ = run(x, wq, bq, wk, bk, wv, bv, wo, bo, trace=False)
    return out


# revision 30
# speedup vs baseline: 1.0297x; 1.0297x over previous
"""BlockAttention TRN2 Bass kernel (algebraically fused, all-fp16 matmuls).

Problem (hardcoded): x [4, 4096, 1024] fp32; wq/wk/wv/wo [1024, 1024];
bq/bk/bv/bo [1024]; block_size 256. Output [4, 8192, 1024]:
per 256-token block g: rows [512g, 512g+256) = softmax(Q_g K_g^T / 32) V_g @ wo,
rows [512g+256, 512g+512) = softmax(Q_g K_{g-1}^T / 32) V_{g-1} @ wo (block 0
attends to itself), all + bo.

Sharding: 8 cores = 4 batches x 2 sequence halves (8 q-blocks each). Each core
gets x^T for its 9 kv blocks (prev + 8 own; block 0's "prev" is itself), the
fused weights, and writes out^T [1024, 4096] bf16 for its 4096 output rows.

Algebraic fusion (host precomputes, exact):
  - Aqk = wq @ wk^T: scores S[q,k] = x_q^T Aqk x_k (+ per-key bias term
    kb[k] = x_k.(wk bq) + bq.bk, softmax-invariant per-query terms dropped).
    One projection QA = (x @ Aqk)^T replaces the Q and K projections, and the
    score matmuls contract QA against the raw x^T tiles already in SBUF.
  - Awv = wv @ wo: VW = x @ Awv replaces V-proj + V@wo. The bv part:
    P_norm @ (1 bv^T wo) = bv@wo (softmax rows sum to 1), added on host
    with bo.

Why fp16 everywhere (measured on this data): fp8e4m3 DoubleRow is 2x the
matmul rate, but at concentrated-attention queries (max softmax weight ~0.4)
a 4% fp8 quantization of VW or P lands directly on the output: ~4e-2 rel
error vs the 2e-2 budget. fp16 matmuls (full PE rate, 10-bit mantissa) give
2.6e-3. PSUM accumulation is fp32 throughout. Accumulation groups sharing a
PSUM bank always run back-to-back; interleaving two open groups in one bank
corrupts the first group's start on hardware.

Per-core algorithm:
  - QA^T = Aqk^T x^T per q-block PAIR (512-wide moving operand amortizes
    the per-matmul weight-load bubble).
  - VW = x @ Awv per kv block ([token-partitions, d_out] layout), reused by
    the local attention of block g and the cross attention of block g+1.
  - S^T [keys, queries] = X_kv QA^T (no transposes anywhere); softmax over
    the partition (key) dim: exp on ScalarE (per-key bias, fp16 out),
    key-sums via ones-vector matmul, reciprocal on VectorE, normalization
    deferred to the PSUM->SBUF output copy (diag scaling commutes with V@wo).
  - out^T = VW^T P^T in PSUM, staged [128, 512] bf16 per (m, q-block) so
    local|cross form one contiguous DMA row pair, then straight to DRAM.
"""

import numpy as np
import ml_dtypes
from contextlib import ExitStack

import concourse.bass as bass
import concourse.mybir as mybir
import concourse.tile as tile
from concourse import bacc, bass_utils

D = 1024
BS = 256
NBQ = 8  # q-blocks per core
NKV = NBQ + 1  # kv blocks (prev + own 8)
DS = D // 128  # 8 subtiles of the feature dim
F32 = mybir.dt.float32
F32R = mybir.dt.float32r
BF16 = mybir.dt.bfloat16
FP16 = mybir.dt.float16
SCALE = 1.0 / 32.0  # 1/sqrt(D)

_CACHED_NC = None


def _build():
    nc = bacc.Bacc("TRN2", target_bir_lowering=False, debug=False, num_devices=8)
    # block-tiled fp16 x^T: row 128*b+p, col 256*s+c = x_kv[256b+c, 128s+p]
    x16 = nc.dram_tensor("x16", [NKV * 128, DS * BS], FP16, kind="ExternalInput").ap()
    aqk = nc.dram_tensor("aqk", [128, DS * D], FP16, kind="ExternalInput").ap()
    awv = nc.dram_tensor("awv", [128, DS * D], FP16, kind="ExternalInput").ap()
    kb = nc.dram_tensor("kb", [128, NKV * 2], F32, kind="ExternalInput").ap()
    ones16 = nc.dram_tensor("ones16", [128, 128], FP16, kind="ExternalInput").ap()
    outt = nc.dram_tensor("outt", [D, NBQ * 2 * BS], BF16, kind="ExternalOutput").ap()

    with (
        tile.TileContext(nc) as tc,
        ExitStack() as ctx,
        nc.allow_low_precision(reason="fp16 matmul inputs by design"),
    ):
        wp = ctx.enter_context(tc.tile_pool(name="wp", bufs=1))
        cp = ctx.enter_context(tc.tile_pool(name="cp", bufs=1))
        xp16 = ctx.enter_context(tc.tile_pool(name="xp16", bufs=3))
        qp = ctx.enter_context(tc.tile_pool(name="qp", bufs=2))
        wvp = ctx.enter_context(tc.tile_pool(name="wvp", bufs=3))
        pp = ctx.enter_context(tc.tile_pool(name="pp", bufs=4))
        rp = ctx.enter_context(tc.tile_pool(name="rp", bufs=4))
        sp_ = ctx.enter_context(tc.tile_pool(name="sp", bufs=16))
        PSUM = bass.MemorySpace.PSUM
        ps = ctx.enter_context(tc.tile_pool(name="ps", bufs=8, space=PSUM))

        # fused fp16 weights, [128, DS, D]: [p, s, d] = W[128s+p, d].
        # Split into two DMAs across two rings to halve arrival latency.
        w_sb = {}

        def load_w(name, ap, npkt=2):
            # npkt packets alternating across the two hardware rings; finer
            # packets shrink the region a consumer matmul must wait for.
            t = wp.tile([128, DS, D], FP16, tag=name)
            step = DS // npkt
            for i in range(npkt):
                eng = nc.sync if i % 2 == 0 else nc.scalar
                eng.dma_start(
                    t[:, step * i : step * (i + 1), :],
                    ap[:, step * i * D : step * (i + 1) * D],
                )
            w_sb[name] = t

        def load_x16(b0, nb, engines=None):
            # fp16 x^T tile [128, DS, 2*BS] holding blocks b0, b0+1 (nb=2)
            # or just b0 (nb=1); one 512KB DMA per block, 2KB rows. Steady
            # loads ride the gpsimd ring; startup-critical ones are placed
            # on the fast hardware rings via `engines`.
            t = xp16.tile([128, DS, 2 * BS], FP16, tag="x16")
            for i in range(nb):
                eng = engines[i] if engines else nc.gpsimd
                eng.dma_start(
                    t[:, :, BS * i : BS * (i + 1)],
                    x16[128 * (b0 + i) : 128 * (b0 + i + 1), :],
                )
            return t

        def qa_proj(xpair):
            # (Aqk^T x^T)[d_out, tok] fp16 for a q-block pair: [128, DS, 2*BS],
            # d_out-subtile m at [:, m, :]; 512-wide moving operand.
            dst = qp.tile([128, DS, 2 * BS], FP16, tag="qa")
            for m in range(DS):
                pst = ps.tile([128, 512], F32, tag="ps")
                for s in range(DS):
                    nc.tensor.matmul(
                        pst[:],
                        w_sb["aqk"][:, s, 128 * m : 128 * (m + 1)],
                        xpair[:, s, :],
                        start=(s == 0),
                        stop=(s == DS - 1),
                    )
                nc.scalar.activation(
                    dst[:, m, :],
                    pst[:],
                    mybir.ActivationFunctionType.Identity,
                )
            return dst

        def vw_direct(xtile, kc):
            # (x @ Awv)[tok, d_out] fp16 for block half kc of the pair tile:
            # [128, 2, D]: [p, ts, d] = VW[128*ts+p, d]; 512-wide moving.
            dst = wvp.tile([128, 2, D], FP16, tag="vw")
            for ts in range(2):
                pst = [
                    ps.tile([128, 512], F32, tag="ps", name=f"pst{i}")
                    for i in range(2)
                ]
                for half in range(2):
                    for s in range(DS):
                        nc.tensor.matmul(
                            pst[half][:],
                            xtile[:, s, BS * kc + 128 * ts : BS * kc + 128 * (ts + 1)],
                            w_sb["awv"][:, s, 512 * half : 512 * (half + 1)],
                            start=(s == 0),
                            stop=(s == DS - 1),
                        )
                for half in range(2):
                    # on ScalarE: keeps VectorE free for the reciprocals and
                    # output muls, so attend_out never waits on these casts
                    nc.scalar.activation(
                        dst[:, ts, 512 * half : 512 * (half + 1)],
                        pst[half][:],
                        mybir.ActivationFunctionType.Identity,
                    )
            return dst

        def attend_scores(qa, qc, xkv, kc, kvblk):
            # expS^T = exp(K Q^T / 32 + kbias) fp16, unnormalized.
            # [128, 2, BS]: key-half ks at [:, ks, :]. lhsT = raw x^T key
            # slices from half kc of the x16 pair tile, rhs = qa half qc.
            pst = ps.tile([128, 512], F32, tag="ps")
            ptile = pp.tile([128, 2, BS], FP16, tag="pt")
            for ks in range(2):
                for s in range(DS):
                    nc.tensor.matmul(
                        pst[:, BS * ks : BS * (ks + 1)],
                        xkv[:, s, BS * kc + 128 * ks : BS * kc + 128 * (ks + 1)],
                        qa[:, s, BS * qc : BS * (qc + 1)],
                        start=(s == 0),
                        stop=(s == DS - 1),
                    )
                c = 2 * kvblk + ks
                nc.scalar.activation(
                    ptile[:, ks, :],
                    pst[:, BS * ks : BS * (ks + 1)],
                    mybir.ActivationFunctionType.Exp,
                    scale=SCALE,
                    bias=kb_sb[:, c : c + 1],
                )
            return ptile

        def attend_norm(pt, bc, i):
            # Broadcasted key-sum of one attend into half a shared PSUM bank
            # (sequential groups); 128-lane reciprocal on VectorE. Emitted
            # right after the attend's scores so the slow reciprocal drains
            # under the following PE phases — by the time the output muls
            # need rc, it's long ready and PSUM slots recycle instantly.
            for ks in range(2):
                nc.tensor.matmul(
                    bc[:, BS * i : BS * (i + 1)],
                    ones_sb[:],
                    pt[:, ks, :],
                    start=(ks == 0),
                    stop=(ks == 1),
                )
            # rc doubled along a middle dim so one wide mul per PSUM tile can
            # normalize both d_out subtiles at once (faster PSUM recycling)
            rc = rp.tile([128, 2, BS], F32R, tag="rc")
            nc.vector.reciprocal(rc[:, 0, :], bc[:, BS * i : BS * (i + 1)])
            nc.vector.tensor_copy(rc[:, 1, :], rc[:, 0, :])
            return rc

        def attend_out(ptile, rc, vw, t, h, stages, drain_half=False):
            # out^T[m-subtile, q] = VW^T P^T (256-deep key contraction in two
            # ks steps). Normalize on the PSUM->SBUF copy into the [128, 512]
            # bf16 stage (local h=0 left, cross h=1 right); the stage row
            # pair is one contiguous DMA per m.
            for mp in range(DS // 2):
                pso = ps.tile([128, 2, BS], F32, tag="ps")
                for sub in range(2):
                    m = 2 * mp + sub
                    for ks in range(2):
                        nc.tensor.matmul(
                            pso[:, sub, :],
                            vw[:, ks, 128 * m : 128 * (m + 1)],
                            ptile[:, ks, :],
                            start=(ks == 0),
                            stop=(ks == 1),
                        )
                # one wide mul normalizes both subtiles into the stage's
                # h-plane (strided dst), freeing the PSUM tile in one shot
                nc.vector.tensor_mul(stages[mp][:, :, h, :], pso[:], rc[:])
                for sub in range(2):
                    m = 2 * mp + sub
                    eng = nc.sync if m % 2 else nc.scalar
                    if drain_half:  # final iteration: don't hold the tail
                        eng.dma_start(
                            outt[
                                128 * m : 128 * (m + 1),
                                512 * t + BS * h : 512 * t + BS * (h + 1),
                            ],
                            stages[mp][:, sub, h, :],
                        )
                    elif h == 0:  # local runs second; both halves written
                        eng.dma_start(
                            outt[128 * m : 128 * (m + 1), 512 * t : 512 * (t + 1)],
                            stages[mp][:, sub, :, :],
                        )

        # Prologue DMAs on the two fast hardware rings (~110 GB/s each),
        # interleaved per d-subtile in consumption order: the first qa
        # matmul needs only subtile 0 of aqk and of x16 blocks 1+2, so the
        # PE starts as soon as the first three small packets land and rides
        # just behind the rings through the rest. Block 0 (cross keys +
        # vw_prev) and awv are only touched ~20us in. The gpsimd software
        # ring (slow, ~90 GB/s) carries constants + steady x16 prefetch.
        aqk_t = wp.tile([128, DS, D], FP16, tag="aqk")
        w_sb["aqk"] = aqk_t
        x16_cur = xp16.tile([128, DS, 2 * BS], FP16, tag="x16")  # q-blocks 1,2
        for s in range(DS):
            e0, e1 = (nc.sync, nc.scalar) if s % 2 == 0 else (nc.scalar, nc.sync)
            e0.dma_start(aqk_t[:, s, :], aqk[:, s * D : (s + 1) * D])
            e1.dma_start(x16_cur[:, s, 0:BS], x16[128 * 1 : 128 * 2, 256 * s : 256 * (s + 1)])
            # block 2 arrives slightly later than block 1 is consumed; the
            # otherwise-idle gpsimd ring keeps it off the two fast rings
            nc.gpsimd.dma_start(
                x16_cur[:, s, BS : 2 * BS], x16[128 * 2 : 128 * 3, 256 * s : 256 * (s + 1)]
            )
        kb_sb = cp.tile([128, NKV * 2], F32, tag="kb")
        nc.gpsimd.dma_start(kb_sb[:], kb)
        ones_sb = cp.tile([128, 128], FP16, tag="ones")
        nc.gpsimd.dma_start(ones_sb[:], ones16)
        x16_prev = load_x16(0, 1, engines=(nc.scalar,))  # kv block 0
        load_w("awv", awv, npkt=4)

        vw_prev = None
        qa = None
        for t in range(NBQ):
            b = t + 1  # kv block holding this q-block's tokens
            qc = t % 2  # column half within the qa / x16 pair
            if qc == 0:
                qa = qa_proj(x16_cur)
            x16_next = (
                load_x16(b + 1, 2 if b + 2 <= NBQ else 1)
                if qc == 1 and b + 1 <= NBQ
                else None
            )
            bc = ps.tile([128, 512], F32, tag="ps")
            p_loc = attend_scores(qa, qc, x16_cur, qc, b)
            # cross keys: block b-1 = left half of this pair tile (odd t),
            # right half of the previous pair tile (even t>0), or the lone
            # block-0 tile (t=0, which arrives last — all earlier PE work
            # stays off it and off awv).
            if qc == 1:
                p_cross = attend_scores(qa, qc, x16_cur, 0, b - 1)
            else:
                p_cross = attend_scores(qa, qc, x16_prev, 1 if t else 0, b - 1)
            rc_loc = attend_norm(p_loc, bc, 0)
            rc_cross = attend_norm(p_cross, bc, 1)
            vw_cur = vw_direct(x16_cur, qc)
            if t == 0:
                vw_prev = vw_direct(x16_prev, 0)
            # per m-pair: [p, m-sub, local|cross, q] — each m's local|cross
            # row pair stays one contiguous DMA
            stages = [
                sp_.tile([128, 2, 2, BS], BF16, tag="st", name=f"stage{m}")
                for m in range(DS // 2)
            ]
            # cross first: its vw_prev is ready from last iteration, so the
            # PE stream never waits on vw_cur's PSUM->SBUF copies.
            last = t == NBQ - 1
            attend_out(p_cross, rc_cross, vw_prev, t, 1, stages, drain_half=last)
            attend_out(p_loc, rc_loc, vw_cur, t, 0, stages, drain_half=last)
            vw_prev = vw_cur
            if qc == 1:
                x16_prev, x16_cur = x16_cur, x16_next

    nc.compile()
    return nc


def _get_nc():
    global _CACHED_NC
    if _CACHED_NC is None:
        _CACHED_NC = _build()
    return _CACHED_NC


def _make_in_maps(x, wq, bq, wk, bk, wv, bv, wo):
    aqk = (wq @ wk.T).astype(np.float32)
    awv = (wv @ wo).astype(np.float32)
    # weight tiles [128, DS*D]: [p, s*D+d] = W[128s+p, d]
    wtile = lambda w: np.ascontiguousarray(
        w.reshape(DS, 128, D).transpose(1, 0, 2).reshape(128, DS * D),
        np.float16,
    )
    # per-key score bias (exact; zero when bq == 0): kb[tok] = x.(wk bq)+bq.bk
    kbv = (wk @ bq).astype(np.float32)
    kb_full = (x.reshape(-1, D) @ kbv + float(bq @ bk)).reshape(4, -1) * SCALE
    base = {
        "aqk": wtile(aqk),
        "awv": wtile(awv),
        "ones16": np.ones((128, 128), np.float16),
    }
    in_maps = []
    for c in range(8):
        b, t = c // 2, c % 2
        if t == 0:
            xkv = np.concatenate([x[b, 0:BS], x[b, 0 : NBQ * BS]], axis=0)
            kbc = np.concatenate([kb_full[b, 0:BS], kb_full[b, 0 : NBQ * BS]])
        else:
            xkv = x[b, NBQ * BS - BS : 2 * NBQ * BS]
            kbc = kb_full[b, NBQ * BS - BS : 2 * NBQ * BS]
        # block-tiled: row 128*b+p, col 256*s+c = xkv[256*blk+c, 128*s+p]
        in_maps.append(
            {
                **base,
                "x16": np.ascontiguousarray(
                    xkv.reshape(NKV, BS, DS, 128)
                    .transpose(0, 3, 2, 1)
                    .reshape(NKV * 128, DS * BS),
                    np.float16,
                ),
                "kb": np.ascontiguousarray(
                    kbc.reshape(NKV * 2, 128).T, np.float32
                ),
            }
        )
    return in_maps


def _assemble(results, bv, wo, bo):
    out = np.empty((4, 2 * NBQ * 2 * BS, D), np.float32)
    for c in range(8):
        b, t = c // 2, c % 2
        seg = NBQ * 2 * BS  # 4096 output rows per core
        out[b, seg * t : seg * (t + 1), :] = results[c]["outt"].T.astype(np.float32)
    out += (np.asarray(bo, np.float32) + bv @ wo).reshape(1, 1, D)
    return out


def run(x, wq, bq, wk, bk, wv, bv, wo, bo, trace=False):
    nc = _get_nc()
    in_maps = _make_in_maps(x, wq, bq, wk, bk, wv, bv, wo)
    res = bass_utils.run_bass_kernel_spmd(
        nc, in_maps, core_ids=list(range(8)), trace=trace
    )
    return _assemble(res.results, bv, wo, bo), res


def kernel(x, wq, bq, wk, bk, wv, bv, wo, bo, block_size):
    assert int(block_size) == BS
    x = np.asarray(x, np.float32)
    assert x.shape == (4, 2 * NBQ * BS, D), x.shape
    args = [np.asarray(a, np.float32) for a in (wq, bq, wk, bk, wv, bv, wo, bo)]
    wq, bq, wk, bk, wv, bv, wo, bo = args
    out, _ = run(x, wq, bq, wk, bk, wv, bv, wo, bo, trace=False)
    return out


# revision 31
# speedup vs baseline: 1.0385x; 1.0086x over previous
"""BlockAttention TRN2 Bass kernel (algebraically fused, all-fp16 matmuls).

Problem (hardcoded): x [4, 4096, 1024] fp32; wq/wk/wv/wo [1024, 1024];
bq/bk/bv/bo [1024]; block_size 256. Output [4, 8192, 1024]:
per 256-token block g: rows [512g, 512g+256) = softmax(Q_g K_g^T / 32) V_g @ wo,
rows [512g+256, 512g+512) = softmax(Q_g K_{g-1}^T / 32) V_{g-1} @ wo (block 0
attends to itself), all + bo.

Sharding: 8 cores = 4 batches x 2 sequence halves (8 q-blocks each). Each core
gets x^T for its 9 kv blocks (prev + 8 own; block 0's "prev" is itself), the
fused weights, and writes out^T [1024, 4096] bf16 for its 4096 output rows.

Algebraic fusion (host precomputes, exact):
  - Aqk = wq @ wk^T: scores S[q,k] = x_q^T Aqk x_k (+ per-key bias term
    kb[k] = x_k.(wk bq) + bq.bk, softmax-invariant per-query terms dropped).
    One projection QA = (x @ Aqk)^T replaces the Q and K projections, and the
    score matmuls contract QA against the raw x^T tiles already in SBUF.
  - Awv = wv @ wo: VW = x @ Awv replaces V-proj + V@wo. The bv part:
    P_norm @ (1 bv^T wo) = bv@wo (softmax rows sum to 1), added on host
    with bo.

Why fp16 everywhere (measured on this data): fp8e4m3 DoubleRow is 2x the
matmul rate, but at concentrated-attention queries (max softmax weight ~0.4)
a 4% fp8 quantization of VW or P lands directly on the output: ~4e-2 rel
error vs the 2e-2 budget. fp16 matmuls (full PE rate, 10-bit mantissa) give
2.6e-3. PSUM accumulation is fp32 throughout. Accumulation groups sharing a
PSUM bank always run back-to-back; interleaving two open groups in one bank
corrupts the first group's start on hardware.

Per-core algorithm:
  - QA^T = Aqk^T x^T per q-block PAIR (512-wide moving operand amortizes
    the per-matmul weight-load bubble).
  - VW = x @ Awv per kv block ([token-partitions, d_out] layout), reused by
    the local attention of block g and the cross attention of block g+1.
  - S^T [keys, queries] = X_kv QA^T (no transposes anywhere); softmax over
    the partition (key) dim: exp on ScalarE (per-key bias, fp16 out),
    key-sums via ones-vector matmul, reciprocal on VectorE, normalization
    deferred to the PSUM->SBUF output copy (diag scaling commutes with V@wo).
  - out^T = VW^T P^T in PSUM, staged [128, 512] bf16 per (m, q-block) so
    local|cross form one contiguous DMA row pair, then straight to DRAM.
"""

import numpy as np
import ml_dtypes
from contextlib import ExitStack

import concourse.bass as bass
import concourse.mybir as mybir
import concourse.tile as tile
from concourse import bacc, bass_utils

D = 1024
BS = 256
NBQ = 8  # q-blocks per core
NKV = NBQ + 1  # kv blocks (prev + own 8)
DS = D // 128  # 8 subtiles of the feature dim
F32 = mybir.dt.float32
F32R = mybir.dt.float32r
BF16 = mybir.dt.bfloat16
FP16 = mybir.dt.float16
SCALE = 1.0 / 32.0  # 1/sqrt(D)

_CACHED_NC = None


def _build():
    nc = bacc.Bacc("TRN2", target_bir_lowering=False, debug=False, num_devices=8)
    # block-tiled fp16 x^T: row 128*b+p, col 256*s+c = x_kv[256b+c, 128s+p]
    x16 = nc.dram_tensor("x16", [NKV * 128, DS * BS], FP16, kind="ExternalInput").ap()
    aqk = nc.dram_tensor("aqk", [128, DS * D], FP16, kind="ExternalInput").ap()
    awv = nc.dram_tensor("awv", [128, DS * D], FP16, kind="ExternalInput").ap()
    kb = nc.dram_tensor("kb", [128, NKV * 2], F32, kind="ExternalInput").ap()
    ones16 = nc.dram_tensor("ones16", [128, 128], FP16, kind="ExternalInput").ap()
    outt = nc.dram_tensor("outt", [D, NBQ * 2 * BS], BF16, kind="ExternalOutput").ap()

    with (
        tile.TileContext(nc) as tc,
        ExitStack() as ctx,
        nc.allow_low_precision(reason="fp16 matmul inputs by design"),
    ):
        wp = ctx.enter_context(tc.tile_pool(name="wp", bufs=1))
        cp = ctx.enter_context(tc.tile_pool(name="cp", bufs=1))
        xp16 = ctx.enter_context(tc.tile_pool(name="xp16", bufs=4))
        qp = ctx.enter_context(tc.tile_pool(name="qp", bufs=2))
        wvp = ctx.enter_context(tc.tile_pool(name="wvp", bufs=4))
        pp = ctx.enter_context(tc.tile_pool(name="pp", bufs=6))
        rp = ctx.enter_context(tc.tile_pool(name="rp", bufs=6))
        sp_ = ctx.enter_context(tc.tile_pool(name="sp", bufs=16))
        PSUM = bass.MemorySpace.PSUM
        ps = ctx.enter_context(tc.tile_pool(name="ps", bufs=8, space=PSUM))

        # fused fp16 weights, [128, DS, D]: [p, s, d] = W[128s+p, d].
        # Split into two DMAs across two rings to halve arrival latency.
        w_sb = {}

        def load_w(name, ap, npkt=2):
            # npkt packets alternating across the two hardware rings; finer
            # packets shrink the region a consumer matmul must wait for.
            t = wp.tile([128, DS, D], FP16, tag=name)
            step = DS // npkt
            for i in range(npkt):
                eng = nc.sync if i % 2 == 0 else nc.scalar
                eng.dma_start(
                    t[:, step * i : step * (i + 1), :],
                    ap[:, step * i * D : step * (i + 1) * D],
                )
            w_sb[name] = t

        def load_x16(b0, nb, engines=None):
            # fp16 x^T tile [128, DS, 2*BS] holding blocks b0, b0+1 (nb=2)
            # or just b0 (nb=1); one 512KB DMA per block, 2KB rows. Steady
            # loads ride the gpsimd ring; startup-critical ones are placed
            # on the fast hardware rings via `engines`.
            t = xp16.tile([128, DS, 2 * BS], FP16, tag="x16")
            for i in range(nb):
                eng = engines[i] if engines else nc.gpsimd
                eng.dma_start(
                    t[:, :, BS * i : BS * (i + 1)],
                    x16[128 * (b0 + i) : 128 * (b0 + i + 1), :],
                )
            return t

        def qa_proj(xpair):
            # (Aqk^T x^T)[d_out, tok] fp16 for a q-block pair: [128, DS, 2*BS],
            # d_out-subtile m at [:, m, :]; 512-wide moving operand.
            dst = qp.tile([128, DS, 2 * BS], FP16, tag="qa")
            for m in range(DS):
                pst = ps.tile([128, 512], F32, tag="ps")
                for s in range(DS):
                    nc.tensor.matmul(
                        pst[:],
                        w_sb["aqk"][:, s, 128 * m : 128 * (m + 1)],
                        xpair[:, s, :],
                        start=(s == 0),
                        stop=(s == DS - 1),
                    )
                nc.scalar.activation(
                    dst[:, m, :],
                    pst[:],
                    mybir.ActivationFunctionType.Identity,
                )
            return dst

        def vw_direct(xtile, kc):
            # (x @ Awv)[tok, d_out] fp16 for block half kc of the pair tile:
            # [128, 2, D]: [p, ts, d] = VW[128*ts+p, d]; 512-wide moving.
            dst = wvp.tile([128, 2, D], FP16, tag="vw")
            for ts in range(2):
                pst = [
                    ps.tile([128, 512], F32, tag="ps", name=f"pst{i}")
                    for i in range(2)
                ]
                for half in range(2):
                    for s in range(DS):
                        nc.tensor.matmul(
                            pst[half][:],
                            xtile[:, s, BS * kc + 128 * ts : BS * kc + 128 * (ts + 1)],
                            w_sb["awv"][:, s, 512 * half : 512 * (half + 1)],
                            start=(s == 0),
                            stop=(s == DS - 1),
                        )
                for half in range(2):
                    # on ScalarE: keeps VectorE free for the reciprocals and
                    # output muls, so attend_out never waits on these casts
                    nc.scalar.activation(
                        dst[:, ts, 512 * half : 512 * (half + 1)],
                        pst[half][:],
                        mybir.ActivationFunctionType.Identity,
                    )
            return dst

        def attend_scores(qa, qc, xkv, kc, kvblk):
            # expS^T = exp(K Q^T / 32 + kbias) fp16, unnormalized.
            # [128, 2, BS]: key-half ks at [:, ks, :]. lhsT = raw x^T key
            # slices from half kc of the x16 pair tile, rhs = qa half qc.
            pst = ps.tile([128, 512], F32, tag="ps")
            ptile = pp.tile([128, 2, BS], FP16, tag="pt")
            for ks in range(2):
                for s in range(DS):
                    nc.tensor.matmul(
                        pst[:, BS * ks : BS * (ks + 1)],
                        xkv[:, s, BS * kc + 128 * ks : BS * kc + 128 * (ks + 1)],
                        qa[:, s, BS * qc : BS * (qc + 1)],
                        start=(s == 0),
                        stop=(s == DS - 1),
                    )
                c = 2 * kvblk + ks
                nc.scalar.activation(
                    ptile[:, ks, :],
                    pst[:, BS * ks : BS * (ks + 1)],
                    mybir.ActivationFunctionType.Exp,
                    scale=SCALE,
                    bias=kb_sb[:, c : c + 1],
                )
            return ptile

        def attend_norm(pt, bc, i):
            # Broadcasted key-sum of one attend into half a shared PSUM bank
            # (sequential groups); 128-lane reciprocal on VectorE. Emitted
            # right after the attend's scores so the slow reciprocal drains
            # under the following PE phases — by the time the output muls
            # need rc, it's long ready and PSUM slots recycle instantly.
            for ks in range(2):
                nc.tensor.matmul(
                    bc[:, BS * i : BS * (i + 1)],
                    ones_sb[:],
                    pt[:, ks, :],
                    start=(ks == 0),
                    stop=(ks == 1),
                )
            # rc doubled along a middle dim so one wide mul per PSUM tile can
            # normalize both d_out subtiles at once (faster PSUM recycling)
            rc = rp.tile([128, 2, BS], F32R, tag="rc")
            nc.vector.reciprocal(rc[:, 0, :], bc[:, BS * i : BS * (i + 1)])
            nc.vector.tensor_copy(rc[:, 1, :], rc[:, 0, :])
            return rc

        def attend_out(ptile, rc, vw, t, h, stages, drain_half=False):
            # out^T[m-subtile, q] = VW^T P^T (256-deep key contraction in two
            # ks steps). Normalize on the PSUM->SBUF copy into the [128, 512]
            # bf16 stage (local h=0 left, cross h=1 right); the stage row
            # pair is one contiguous DMA per m.
            for mp in range(DS // 2):
                pso = ps.tile([128, 2, BS], F32, tag="ps")
                for sub in range(2):
                    m = 2 * mp + sub
                    for ks in range(2):
                        nc.tensor.matmul(
                            pso[:, sub, :],
                            vw[:, ks, 128 * m : 128 * (m + 1)],
                            ptile[:, ks, :],
                            start=(ks == 0),
                            stop=(ks == 1),
                        )
                # one wide mul normalizes both subtiles into the stage's
                # h-plane (strided dst), freeing the PSUM tile in one shot
                nc.vector.tensor_mul(stages[mp][:, :, h, :], pso[:], rc[:])
                for sub in range(2):
                    m = 2 * mp + sub
                    eng = nc.sync if m % 2 else nc.scalar
                    if drain_half:  # final iteration: don't hold the tail
                        eng.dma_start(
                            outt[
                                128 * m : 128 * (m + 1),
                                512 * t + BS * h : 512 * t + BS * (h + 1),
                            ],
                            stages[mp][:, sub, h, :],
                        )
                    elif h == 0:  # local runs second; both halves written
                        eng.dma_start(
                            outt[128 * m : 128 * (m + 1), 512 * t : 512 * (t + 1)],
                            stages[mp][:, sub, :, :],
                        )

        # Prologue DMAs on the two fast hardware rings (~110 GB/s each),
        # interleaved per d-subtile in consumption order: the first qa
        # matmul needs only subtile 0 of aqk and of x16 blocks 1+2, so the
        # PE starts as soon as the first three small packets land and rides
        # just behind the rings through the rest. Block 0 (cross keys +
        # vw_prev) and awv are only touched ~20us in. The gpsimd software
        # ring (slow, ~90 GB/s) carries constants + steady x16 prefetch.
        aqk_t = wp.tile([128, DS, D], FP16, tag="aqk")
        w_sb["aqk"] = aqk_t
        x16_cur = xp16.tile([128, DS, 2 * BS], FP16, tag="x16")  # q-blocks 1,2
        for s in range(DS):
            e0, e1 = (nc.sync, nc.scalar) if s % 2 == 0 else (nc.scalar, nc.sync)
            e0.dma_start(aqk_t[:, s, :], aqk[:, s * D : (s + 1) * D])
            e1.dma_start(x16_cur[:, s, 0:BS], x16[128 * 1 : 128 * 2, 256 * s : 256 * (s + 1)])
            # block 2 arrives slightly later than block 1 is consumed; the
            # otherwise-idle gpsimd ring keeps it off the two fast rings
            nc.gpsimd.dma_start(
                x16_cur[:, s, BS : 2 * BS], x16[128 * 2 : 128 * 3, 256 * s : 256 * (s + 1)]
            )
        kb_sb = cp.tile([128, NKV * 2], F32, tag="kb")
        nc.gpsimd.dma_start(kb_sb[:], kb)
        ones_sb = cp.tile([128, 128], FP16, tag="ones")
        nc.gpsimd.dma_start(ones_sb[:], ones16)
        x16_prev = load_x16(0, 1, engines=(nc.scalar,))  # kv block 0
        load_w("awv", awv, npkt=4)

        vw_prev = None
        qa = None
        for t in range(NBQ):
            b = t + 1  # kv block holding this q-block's tokens
            qc = t % 2  # column half within the qa / x16 pair
            if qc == 0:
                qa = qa_proj(x16_cur)
            x16_next = (
                load_x16(b + 1, 2 if b + 2 <= NBQ else 1)
                if qc == 1 and b + 1 <= NBQ
                else None
            )
            bc = ps.tile([128, 512], F32, tag="ps")
            p_loc = attend_scores(qa, qc, x16_cur, qc, b)
            # cross keys: block b-1 = left half of this pair tile (odd t),
            # right half of the previous pair tile (even t>0), or the lone
            # block-0 tile (t=0, which arrives last — all earlier PE work
            # stays off it and off awv).
            if qc == 1:
                p_cross = attend_scores(qa, qc, x16_cur, 0, b - 1)
            else:
                p_cross = attend_scores(qa, qc, x16_prev, 1 if t else 0, b - 1)
            rc_loc = attend_norm(p_loc, bc, 0)
            rc_cross = attend_norm(p_cross, bc, 1)
            vw_cur = vw_direct(x16_cur, qc)
            if t == 0:
                vw_prev = vw_direct(x16_prev, 0)
            # per m-pair: [p, m-sub, local|cross, q] — each m's local|cross
            # row pair stays one contiguous DMA
            stages = [
                sp_.tile([128, 2, 2, BS], BF16, tag="st", name=f"stage{m}")
                for m in range(DS // 2)
            ]
            # cross first: its vw_prev is ready from last iteration, so the
            # PE stream never waits on vw_cur's PSUM->SBUF copies.
            last = t == NBQ - 1
            attend_out(p_cross, rc_cross, vw_prev, t, 1, stages, drain_half=last)
            attend_out(p_loc, rc_loc, vw_cur, t, 0, stages, drain_half=last)
            vw_prev = vw_cur
            if qc == 1:
                x16_prev, x16_cur = x16_cur, x16_next

    nc.compile()
    return nc


def _get_nc():
    global _CACHED_NC
    if _CACHED_NC is None:
        _CACHED_NC = _build()
    return _CACHED_NC


def _make_in_maps(x, wq, bq, wk, bk, wv, bv, wo):
    aqk = (wq @ wk.T).astype(np.float32)
    awv = (wv @ wo).astype(np.float32)
    # weight tiles [128, DS*D]: [p, s*D+d] = W[128s+p, d]
    wtile = lambda w: np.ascontiguousarray(
        w.reshape(DS, 128, D).transpose(1, 0, 2).reshape(128, DS * D),
        np.float16,
    )
    # per-key score bias (exact; zero when bq == 0): kb[tok] = x.(wk bq)+bq.bk
    kbv = (wk @ bq).astype(np.float32)
    kb_full = (x.reshape(-1, D) @ kbv + float(bq @ bk)).reshape(4, -1) * SCALE
    base = {
        "aqk": wtile(aqk),
        "awv": wtile(awv),
        "ones16": np.ones((128, 128), np.float16),
    }
    in_maps = []
    for c in range(8):
        b, t = c // 2, c % 2
        if t == 0:
            xkv = np.concatenate([x[b, 0:BS], x[b, 0 : NBQ * BS]], axis=0)
            kbc = np.concatenate([kb_full[b, 0:BS], kb_full[b, 0 : NBQ * BS]])
        else:
            xkv = x[b, NBQ * BS - BS : 2 * NBQ * BS]
            kbc = kb_full[b, NBQ * BS - BS : 2 * NBQ * BS]
        # block-tiled: row 128*b+p, col 256*s+c = xkv[256*blk+c, 128*s+p]
        in_maps.append(
            {
                **base,
                "x16": np.ascontiguousarray(
                    xkv.reshape(NKV, BS, DS, 128)
                    .transpose(0, 3, 2, 1)
                    .reshape(NKV * 128, DS * BS),
                    np.float16,
                ),
                "kb": np.ascontiguousarray(
                    kbc.reshape(NKV * 2, 128).T, np.float32
                ),
            }
        )
    return in_maps


def _assemble(results, bv, wo, bo):
    out = np.empty((4, 2 * NBQ * 2 * BS, D), np.float32)
    for c in range(8):
        b, t = c // 2, c % 2
        seg = NBQ * 2 * BS  # 4096 output rows per core
        out[b, seg * t : seg * (t + 1), :] = results[c]["outt"].T.astype(np.float32)
    out += (np.asarray(bo, np.float32) + bv @ wo).reshape(1, 1, D)
    return out


def run(x, wq, bq, wk, bk, wv, bv, wo, bo, trace=False):
    nc = _get_nc()
    in_maps = _make_in_maps(x, wq, bq, wk, bk, wv, bv, wo)
    res = bass_utils.run_bass_kernel_spmd(
        nc, in_maps, core_ids=list(range(8)), trace=trace
    )
    return _assemble(res.results, bv, wo, bo), res


def kernel(x, wq, bq, wk, bk, wv, bv, wo, bo, block_size):
    assert int(block_size) == BS
    x = np.asarray(x, np.float32)
    assert x.shape == (4, 2 * NBQ * BS, D), x.shape
    args = [np.asarray(a, np.float32) for a in (wq, bq, wk, bk, wv, bv, wo, bo)]
    wq, bq, wk, bk, wv, bv, wo, bo = args
    out, _ = run(x, wq, bq, wk, bk, wv, bv, wo, bo, trace=False)
    return out
